# revision 14
# baseline (speedup 1.0000x reference)
"""DCNv2 (modulated deformable conv) forward on 8 Trainium2 NeuronCores.

Problem: input [4,64,96,96], offset [4,18,96,96], mask [4,9,96,96],
weight [64,64,3,3], bias [64] -> out [4,64,96,96]. STRIDE=1, PAD=1, DIL=1,
deformable groups G=1.

Sharding: data-parallel over (batch, H-half): core = b*2 + h handles output
rows [48h, 48h+48) of batch b; weight/bias replicated.

The gather-bound core loop: per (128-px tile, tap) one indirect DMA (one
row index per dest partition -- the only indirect form this HW supports)
fetches 128 x 256 bf16 from the quad-packed padded image imgPad, where
imgPad[r] = [pixel(r-97) | pixel(r-1)] so a single contiguous 256-bf16 read
at row r = 96*fy+fx+97 returns all four bilinear neighbors
[v00, v10, v01, v11]. imgPad is pure data layout (a shifted/duplicated
transpose of the input image, cast to bf16) built host-side in _host_prep,
like the offset/mask/weight relayouts.

Per-op SWDGE descriptor generation on the Pool/GpSimd engine is the hard
bottleneck: ~1.48 us per indirect DMA regardless of payload (microbench:
324 back-to-back gathers = 479 us; ~1 us fixed Q7 ucode launch per op;
dma_gather/multi-index forms that would amortize it are broken under this
terminal). 324 ops/core => ~480 us floor. Concurrent DVE traffic inflates
the per-op cost (SBUF port contention with Q7 descriptor-ring writes), so
the steady state keeps DVE to ONE small bf16 multiply per tile and the
bilinear REDUCTION is folded into the PE contraction: out[co,p] =
sum_{k,q,c} W4[co,(k,q,c)] * (wq*g)[(k,q,c),p] with W4 = conv weight
replicated over the 4 quad slots (18 exact 128-row chunks, no overlap
trick needed).

Device pipeline:
  A2: per 128-px tile, PE-transpose offset/mask [27,128] -> omT [128,27].
  A3: sampling positions, floor/frac (cast-roundtrip floor), validity,
      bilinear*mask weights wq (bf16), clamped quad-gather indices; all
      via non-2-port DVE forms (tensor_tensor + broadcast const tiles).
      Tiles 0-5 first so gathers start ~8 us in; rest under the stream.
  B:  per tile: 9 indirect gathers (bf16) -> one DVE mult wg = g*wq ->
      18 PE transposes (scalar engine drains PSUM->SBUF bf16) -> 18
      accumulating PE matmuls vs W4 -> +bias (DVE tensor_tensor,
      free-broadcast) -> HWDGE store.

Measured on 8 axon trn2 cores vs fp32 reference: rel err ~2e-3 (bf16
sampling path), HW exec see test log.
"""

import os
import sys
import types
import numpy as np
import ml_dtypes

for _p in ("/opt/trn_rl_repo",):
    if _p not in sys.path and os.path.isdir(_p):
        sys.path.append(_p)

try:
    import antenv.axon_hooks  # noqa: F401
except ImportError:
    _hookmod = types.ModuleType("antenv.axon_hooks")
    _hookmod._hook = None
    _hookmod.set_axon_ntff_profile_hook = lambda h: setattr(_hookmod, "_hook", h)
    _hookmod.get_axon_ntff_profile_hook = lambda: _hookmod._hook
    sys.modules["antenv.axon_hooks"] = _hookmod

B, C, H, W = 4, 64, 96, 96
K = 9
Co = 64
HW = H * W                  # 9216
N_CORES = 8
HHALF = 48
NPIX = HHALF * W            # 4608 output pixels per core
NT = NPIX // 128            # 36 tiles
G0 = 4                      # tiles in the fast-path first group
NCH = K * 2                 # 18 contraction chunks of 128 = (k, l) x (v, c)

_CACHE = {}


def _build_module():
    from contextlib import ExitStack

    import concourse.bass as bass
    import concourse.tile as tile
    from concourse import bacc, mybir
    from concourse.bass_interp import get_hw_module
    from concourse.masks import make_identity

    f32 = mybir.dt.float32
    bf16 = mybir.dt.bfloat16
    i32 = mybir.dt.int32
    Alu = mybir.AluOpType
    Act = mybir.ActivationFunctionType

    nc = bacc.Bacc("TRN2", target_bir_lowering=False, debug=False,
                   enable_asserts=False, num_devices=N_CORES)

    # imgPad[r, 0:64] = pixel(r-97), imgPad[r, 64:128] = pixel(r-1), bf16.
    img_pad_ap = nc.dram_tensor("imgpad", [HW + 98, 2 * C], bf16,
                                kind="ExternalInput").ap()
    offmask_ap = nc.dram_tensor("offmask", [27, NPIX], f32, kind="ExternalInput").ap()
    byx_ap = nc.dram_tensor("byx", [128, NT * K * 2], f32, kind="ExternalInput").ap()
    w4_ap = nc.dram_tensor("w4", [(NCH + 1) * 128, Co], bf16, kind="ExternalInput").ap()
    bias_ap = nc.dram_tensor("biasv", [Co, 1], f32, kind="ExternalInput").ap()
    out_ap = nc.dram_tensor("out", [Co, NPIX], f32, kind="ExternalOutput").ap()

    with tile.TileContext(nc) as tc:
        with ExitStack() as ctx:
            cpool = ctx.enter_context(tc.tile_pool(name="consts", bufs=1))
            prep = ctx.enter_context(tc.tile_pool(name="prep", bufs=1))
            tp_ps = ctx.enter_context(tc.tile_pool(name="tr_ps", bufs=2, space="PSUM"))
            tb_ps = ctx.enter_context(tc.tile_pool(name="trB_ps", bufs=4, space="PSUM"))
            opsum = ctx.enter_context(tc.tile_pool(name="opsum", bufs=2, space="PSUM"))
            gpool = ctx.enter_context(tc.tile_pool(name="gather", bufs=8))
            wgpool = ctx.enter_context(tc.tile_pool(name="wg", bufs=4))
            stpool = ctx.enter_context(tc.tile_pool(name="sampT", bufs=3))
            obpool = ctx.enter_context(tc.tile_pool(name="ob", bufs=3))

            # ---- constants / inputs ----
            om = prep.tile([27, NPIX], f32)
            # first group's offset/mask chunk first: it gates the first gather
            nc.sync.dma_start(out=om[:, 0:G0 * 128],
                              in_=offmask_ap[:, 0:G0 * 128])
            ident = cpool.tile([128, 128], f32)
            make_identity(nc, ident[:])
            ident_bf = cpool.tile([128, 128], bf16)
            make_identity(nc, ident_bf[:])
            byx_sb = cpool.tile([128, NT * K * 2], f32)
            nc.sync.dma_start(out=byx_sb[:], in_=byx_ap)
            nc.sync.dma_start(out=om[:, G0 * 128:NPIX],
                              in_=offmask_ap[:, G0 * 128:NPIX])
            w4_sb = cpool.tile([128, (NCH + 1) * Co], bf16)
            nc.sync.dma_start(
                out=w4_sb[:].rearrange("p (f c) -> p f c", f=NCH + 1),
                in_=w4_ap.rearrange("(f p) c -> p f c", p=128),
            )
            ones_row = cpool.tile([128, 128], bf16)
            nc.vector.memset(ones_row[:], 0.0)
            nc.vector.memset(ones_row[0:1, :], 1.0)

            # DVE ops that can enter 2-port SBUF perf mode (tensor_scalar,
            # tensor_copy/cast) contend with GpSimd SWDGE descriptor writes;
            # steady-state vector work uses tensor_tensor against free-dim
            # broadcast constants instead.
            cst = cpool.tile([128, 8], f32)
            CF0, CF95, CFN1, CF94, CF97, CF9312 = range(6)
            for j, val in ((CF0, 0.0), (CF95, 95.0), (CFN1, -1.0),
                           (CF94, 94.0), (CF97, 97.0), (CF9312, 9312.0)):
                nc.vector.memset(cst[:, j:j + 1], val)
            cst_i = cpool.tile([128, 1], i32)
            nc.vector.memset(cst_i[:], 0)

            def cbc(j, shape):
                t = cst[:, j:j + 1]
                while len(t.shape) < len(shape):
                    t = t.unsqueeze(len(t.shape))
                return t.to_broadcast(list(shape))

            def cbci(shape):
                t = cst_i[:, 0:1]
                while len(t.shape) < len(shape):
                    t = t.unsqueeze(len(t.shape))
                return t.to_broadcast(list(shape))

            # ---- phase A2: offset/mask tile transposes (pixel-major) ----
            omT = prep.tile([128, NT * 27], f32)

            def a2(t0, t1):
                for t in range(t0, t1):
                    pt = tp_ps.tile([128, 27], f32, tag="tr")
                    nc.tensor.transpose(
                        out=pt[:], in_=om[:, t * 128:(t + 1) * 128],
                        identity=ident[:27, :27])
                    nc.scalar.activation(
                        out=omT[:, t * 27:(t + 1) * 27], in_=pt[:], func=Act.Copy)

            # ---- phase A3: index & weight math over tiles [t0, t1) ----
            omT3 = omT[:].rearrange("p (t c) -> p t c", t=NT)
            byx4 = byx_sb[:].rearrange("p (t k s) -> p t k s", t=NT, k=K)

            def t3(name):
                t = prep.tile([128, NT * K], f32, tag=name)
                return t, t[:].rearrange("p (t k) -> p t k", t=NT)

            py, pyv = t3("py")
            px, pxv = t3("px")
            fy, fyv = t3("fy")
            fx, fxv = t3("fx")
            wy, wyv = t3("wy")
            wx, wxv = t3("wx")
            ta, tav = t3("ta")
            tb, tbv = t3("tb")
            ti = prep.tile([128, NT * K], i32, tag="ti")
            tiv = ti[:].rearrange("p (t k) -> p t k", t=NT)
            vm0, vm0v = t3("vm0")
            vm1, vm1v = t3("vm1")
            vc0, vc0v = t3("vc0")
            vc1, vc1v = t3("vc1")
            cA, cAv = t3("cA")
            cB, cBv = t3("cB")
            wq = prep.tile([128, NT * K * 4], bf16)
            wq5 = wq[:].rearrange("p (t k l v) -> p t k l v", t=NT, k=K, l=2)
            idxf = prep.tile([128, NT * K], f32)
            idxi = prep.tile([128, NT * K], i32)
            idxi3 = idxi[:].rearrange("p (t k) -> p t k", t=NT)

            V = nc.vector

            def a3(t0, t1):
                s = slice(t0, t1)
                f = slice(t0 * K, t1 * K)
                dyv = omT3[:, s, 0:18:2]      # [128, nt, 9]
                dxv = omT3[:, s, 1:18:2]
                mv = omT3[:, s, 18:27]
                hov = byx4[:, s, :, 0]        # ho - 1 + ky
                wov = byx4[:, s, :, 1]        # wo - 1 + kx
                _py, _px = pyv[:, s], pxv[:, s]
                _fy, _fx = fyv[:, s], fxv[:, s]
                _wy, _wx = wyv[:, s], wxv[:, s]
                _ta, _tb, _ti = tav[:, s], tbv[:, s], tiv[:, s]
                _vm0, _vm1 = vm0v[:, s], vm1v[:, s]
                _vc0, _vc1 = vc0v[:, s], vc1v[:, s]
                _cA, _cB = cAv[:, s], cBv[:, s]
                shp = list(_py.shape)
                zf = cbc(CF0, shp)

                # py = dy + (ho - 1 + ky); floor & frac (cast-roundtrip floor,
                # robust to any int rounding mode; casts via tensor_tensor
                # add-zero keep DVE in 1-port mode)
                V.tensor_tensor(out=_py, in0=dyv, in1=hov, op=Alu.add)
                V.tensor_tensor(out=_ti, in0=_py, in1=zf, op=Alu.add)
                V.tensor_tensor(out=_ta, in0=_ti, in1=cbci(shp), op=Alu.add)
                V.tensor_tensor(out=_tb, in0=_ta, in1=_py, op=Alu.is_gt)
                V.tensor_tensor(out=_fy, in0=_ta, in1=_tb, op=Alu.subtract)
                V.tensor_tensor(out=_wy, in0=_py, in1=_fy, op=Alu.subtract)
                # px = dx + (wo - 1 + kx)
                V.tensor_tensor(out=_px, in0=dxv, in1=wov, op=Alu.add)
                V.tensor_tensor(out=_ti, in0=_px, in1=zf, op=Alu.add)
                V.tensor_tensor(out=_ta, in0=_ti, in1=cbci(shp), op=Alu.add)
                V.tensor_tensor(out=_tb, in0=_ta, in1=_px, op=Alu.is_gt)
                V.tensor_tensor(out=_fx, in0=_ta, in1=_tb, op=Alu.subtract)
                V.tensor_tensor(out=_wx, in0=_px, in1=_fx, op=Alu.subtract)

                # row validity (* mask) and column validity
                V.tensor_tensor(out=_ta, in0=_fy, in1=zf, op=Alu.is_ge)
                V.tensor_tensor(out=_tb, in0=_fy, in1=cbc(CF95, shp), op=Alu.is_le)
                V.tensor_tensor(out=_vm0, in0=_ta, in1=_tb, op=Alu.mult)
                V.tensor_tensor(out=_vm0, in0=_vm0, in1=mv, op=Alu.mult)
                V.tensor_tensor(out=_ta, in0=_fy, in1=cbc(CFN1, shp), op=Alu.is_ge)
                V.tensor_tensor(out=_tb, in0=_fy, in1=cbc(CF94, shp), op=Alu.is_le)
                V.tensor_tensor(out=_vm1, in0=_ta, in1=_tb, op=Alu.mult)
                V.tensor_tensor(out=_vm1, in0=_vm1, in1=mv, op=Alu.mult)
                V.tensor_tensor(out=_ta, in0=_fx, in1=zf, op=Alu.is_ge)
                V.tensor_tensor(out=_tb, in0=_fx, in1=cbc(CF95, shp), op=Alu.is_le)
                V.tensor_tensor(out=_vc0, in0=_ta, in1=_tb, op=Alu.mult)
                V.tensor_tensor(out=_ta, in0=_fx, in1=cbc(CFN1, shp), op=Alu.is_ge)
                V.tensor_tensor(out=_tb, in0=_fx, in1=cbc(CF94, shp), op=Alu.is_le)
                V.tensor_tensor(out=_vc1, in0=_ta, in1=_tb, op=Alu.mult)

                # bilinear coefficients: cy0/cy1 (carry mask), cx0/cx1
                nc.scalar.activation(out=_ta, in_=_wy, func=Act.Copy, bias=1.0, scale=-1.0)
                V.tensor_tensor(out=_cA, in0=_ta, in1=_vm0, op=Alu.mult)   # cy0
                V.tensor_tensor(out=_cB, in0=_wy, in1=_vm1, op=Alu.mult)   # cy1
                nc.scalar.activation(out=_ta, in_=_wx, func=Act.Copy, bias=1.0, scale=-1.0)
                V.tensor_tensor(out=_vc0, in0=_ta, in1=_vc0, op=Alu.mult)  # cx0
                V.tensor_tensor(out=_vc1, in0=_wx, in1=_vc1, op=Alu.mult)  # cx1

                V.tensor_tensor(out=wq5[:, s, :, 0, 0], in0=_cA, in1=_vc0, op=Alu.mult)
                V.tensor_tensor(out=wq5[:, s, :, 0, 1], in0=_cB, in1=_vc0, op=Alu.mult)
                V.tensor_tensor(out=wq5[:, s, :, 1, 0], in0=_cA, in1=_vc1, op=Alu.mult)
                V.tensor_tensor(out=wq5[:, s, :, 1, 1], in0=_cB, in1=_vc1, op=Alu.mult)

                # quad-gather indices: clamp(96*fy + fx + 97, 0, 9312)
                idxfv = idxf[:].rearrange("p (t k) -> p t k", t=NT)[:, s]
                V.scalar_tensor_tensor(out=idxfv, in0=_fy, scalar=96.0, in1=_fx,
                                       op0=Alu.mult, op1=Alu.add)
                fl = [128, (t1 - t0) * K]
                V.tensor_tensor(out=idxf[:, f], in0=idxf[:, f],
                                in1=cbc(CF97, fl), op=Alu.add)
                V.tensor_tensor(out=idxf[:, f], in0=idxf[:, f],
                                in1=cbc(CF0, fl), op=Alu.max)
                V.tensor_tensor(out=idxf[:, f], in0=idxf[:, f],
                                in1=cbc(CF9312, fl), op=Alu.min)
                V.tensor_tensor(out=idxi[:, f], in0=idxf[:, f],
                                in1=cbc(CF0, fl), op=Alu.add)

            a2(0, G0)
            a3(0, G0)
            a2(G0, 12)
            a3(G0, 12)
            a2(12, NT)
            a3(12, NT)

            wqv_all = wq[:].rearrange("p (t r) -> p t r", t=NT)
            w4v = w4_sb[:].rearrange("p (f c) -> p f c", f=NCH + 1)

            # ---- phase B ----
            for t in range(NT):
                g = gpool.tile([128, K * 4 * C], bf16)
                for k in range(K):
                    nc.gpsimd.indirect_dma_start(
                        out=g[:, k * 4 * C:(k + 1) * 4 * C],
                        out_offset=None,
                        in_=img_pad_ap,
                        in_offset=bass.IndirectOffsetOnAxis(
                            ap=idxi3[:, t, k:k + 1], axis=0),
                    )
                g5 = g[:].rearrange("p (k l v c) -> p k l v c", k=K, l=2, v=2)
                wq_t = wqv_all[:, t, :].rearrange("p (k l v) -> p k l v", k=K, l=2)
                wq_b = wq_t.unsqueeze(4).to_broadcast([128, K, 2, 2, C])
                wg = wgpool.tile([128, K * 4 * C], bf16)
                wg5 = wg[:].rearrange("p (k l v c) -> p k l v c", k=K, l=2, v=2)
                # two mults (taps 0-4 / 5-8): the first can start after 5 of
                # 9 gathers, shortening the post-last-gather tail chain
                V.tensor_tensor(out=wg5[:, 0:5], in0=g5[:, 0:5],
                                in1=wq_b[:, 0:5], op=Alu.mult)
                V.tensor_tensor(out=wg5[:, 5:K], in0=g5[:, 5:K],
                                in1=wq_b[:, 5:K], op=Alu.mult)

                wgT = stpool.tile([128, NCH * 128], bf16)
                for ci in range(NCH):
                    pt = tb_ps.tile([128, 128], bf16, tag="trB")
                    nc.tensor.transpose(out=pt[:], in_=wg[:, ci * 128:(ci + 1) * 128],
                                        identity=ident_bf[:])
                    nc.scalar.activation(
                        out=wgT[:, ci * 128:(ci + 1) * 128], in_=pt[:],
                        func=Act.Copy)

                po = opsum.tile([Co, 128], f32)
                for ci in range(NCH):
                    nc.tensor.matmul(
                        out=po[:], lhsT=w4v[:, ci, :],
                        rhs=wgT[:, ci * 128:(ci + 1) * 128],
                        start=(ci == 0), stop=False)
                nc.tensor.matmul(
                    out=po[:], lhsT=w4v[:, NCH, :], rhs=ones_row[:],
                    start=False, stop=True)

                ob = obpool.tile([Co, 128], f32)
                nc.scalar.activation(out=ob[:], in_=po[:], func=Act.Copy)
                nc.sync.dma_start(out=out_ap[:, t * 128:(t + 1) * 128], in_=ob[:])

    nc.compile()
    nc.m = get_hw_module(nc.m)
    return nc


def _host_prep(input, offset, mask, weight, bias):
    f32 = np.float32
    bf16 = ml_dtypes.bfloat16
    input = np.ascontiguousarray(input, dtype=f32)
    offset = np.ascontiguousarray(offset, dtype=f32)
    mask = np.ascontiguousarray(mask, dtype=f32)
    weight = np.ascontiguousarray(weight, dtype=f32)
    bias = np.ascontiguousarray(bias, dtype=f32)

    # weight [Co, C, 3, 3] -> W4[(k, l, v, c), co] bf16: conv weight
    # replicated over the 4 bilinear quad slots (l = x-side, v = y-side),
    # matching the gathered quad layout [v00,v10 | v01,v11] per tap.
    wr = weight.reshape(Co, C, K)                     # [co, c, k]
    wkc = np.transpose(wr, (2, 1, 0))                 # [k, c, co]
    w4 = np.broadcast_to(wkc[:, None, None, :, :], (K, 2, 2, C, Co))
    w4 = w4.reshape(NCH * 128, Co)
    # chunk NCH: bias as an outer product against a ones-row rhs
    w4b = np.zeros((128, Co), dtype=np.float32)
    w4b[0, :] = bias
    w4 = np.ascontiguousarray(np.concatenate([w4, w4b], axis=0), dtype=bf16)

    biasv = bias.reshape(Co, 1)
    kyv = (np.arange(K, dtype=f32) // 3)
    kxv = (np.arange(K, dtype=f32) % 3)

    pix = np.arange(NPIX).reshape(NT, 128)
    in_maps = []
    imgpads = {}
    for core in range(N_CORES):
        b, h = core // 2, core % 2
        ho0 = h * HHALF
        ho = ho0 + pix // W
        wo = pix % W
        base_y = (ho - 1)[:, :, None] + kyv[None, None, :]   # [NT, 128, K]
        base_x = (wo - 1)[:, :, None] + kxv[None, None, :]
        byx = np.stack([base_y, base_x], axis=-1)            # [NT, 128, K, 2]
        byx = np.ascontiguousarray(
            byx.transpose(1, 0, 2, 3).reshape(128, NT * K * 2), dtype=f32)
        offmask = np.concatenate(
            [offset[b, :, ho0:ho0 + HHALF, :].reshape(18, NPIX),
             mask[b, :, ho0:ho0 + HHALF, :].reshape(K, NPIX)], axis=0)
        # quad-packed padded image: imgPad[r] = [pixel(r-97) | pixel(r-1)];
        # shared between the two cores of a batch.
        if b not in imgpads:
            imgT = input[b].reshape(C, HW).T.astype(bf16)    # [HW, C]
            ip = np.zeros((HW + 98, 2 * C), dtype=bf16)
            ip[97:97 + HW, 0:C] = imgT
            ip[1:1 + HW, C:2 * C] = imgT
            imgpads[b] = ip
        in_maps.append({
            "imgpad": imgpads[b],
            "offmask": np.ascontiguousarray(offmask),
            "byx": byx,
            "w4": w4,
            "biasv": biasv,
        })
    return in_maps


def kernel(input, offset, mask, weight, bias):
    from concourse.bass_utils import run_bass_kernel_spmd

    if "nc" not in _CACHE:
        _CACHE["nc"] = _build_module()
    nc = _CACHE["nc"]

    in_maps = _host_prep(input, offset, mask, weight, bias)
    res = run_bass_kernel_spmd(nc, in_maps, core_ids=list(range(N_CORES)))

    out = np.empty((B, Co, H, W), dtype=np.float32)
    for core in range(N_CORES):
        b, h = core // 2, core % 2
        ho0 = h * HHALF
        out[b, :, ho0:ho0 + HHALF, :] = \
            res.results[core]["out"].reshape(Co, HHALF, W)
    return out


# revision 15
# speedup vs baseline: 1.3420x; 1.3420x over previous
"""DCNv2 (modulated deformable conv) forward on 8 Trainium2 NeuronCores.

Problem: input [4,64,96,96], offset [4,18,96,96], mask [4,9,96,96],
weight [64,64,3,3], bias [64] -> out [4,64,96,96]. STRIDE=1, PAD=1, DIL=1,
deformable groups G=1.

Sharding: data-parallel over (batch, H-half): core = b*2 + h handles output
rows [48h, 48h+48) of batch b; weight/bias replicated.

The gather-bound core loop: per (128-px tile, tap) one indirect DMA (one
row index per dest partition -- the only indirect form this HW supports)
fetches 128 x 256 bf16 from the quad-packed padded image imgPad, where
imgPad[r] = [pixel(r-97) | pixel(r-1)] so a single contiguous 256-bf16 read
at row r = 96*fy+fx+97 returns all four bilinear neighbors
[v00, v10, v01, v11]. imgPad is pure data layout (a shifted/duplicated
transpose of the input image, cast to bf16) built host-side in _host_prep,
like the offset/mask/weight relayouts.

Per-op SWDGE descriptor generation on the Pool/GpSimd engine is the hard
bottleneck: ~1.48 us per indirect DMA regardless of payload (microbench:
324 back-to-back gathers = 479 us; ~1 us fixed Q7 ucode launch per op;
dma_gather/multi-index forms that would amortize it are broken under this
terminal). 324 ops/core => ~480 us floor. Concurrent DVE traffic inflates
the per-op cost (SBUF port contention with Q7 descriptor-ring writes), so
the steady state keeps DVE to ONE small bf16 multiply per tile and the
bilinear REDUCTION is folded into the PE contraction: out[co,p] =
sum_{k,q,c} W4[co,(k,q,c)] * (wq*g)[(k,q,c),p] with W4 = conv weight
replicated over the 4 quad slots (18 exact 128-row chunks, no overlap
trick needed).

Device pipeline:
  A2: per 128-px tile, PE-transpose offset/mask [27,128] -> omT [128,27].
  A3: sampling positions, floor/frac (cast-roundtrip floor), validity,
      bilinear*mask weights wq (bf16), clamped quad-gather indices; all
      via non-2-port DVE forms (tensor_tensor + broadcast const tiles).
      Tiles 0-5 first so gathers start ~8 us in; rest under the stream.
  B:  per tile: 9 indirect gathers (bf16) -> one DVE mult wg = g*wq ->
      18 PE transposes (scalar engine drains PSUM->SBUF bf16) -> 18
      accumulating PE matmuls vs W4 -> +bias (DVE tensor_tensor,
      free-broadcast) -> HWDGE store.

Measured on 8 axon trn2 cores vs fp32 reference: rel err ~2e-3 (bf16
sampling path), HW exec see test log.
"""

import os
import sys
import types
import numpy as np
import ml_dtypes

for _p in ("/opt/trn_rl_repo",):
    if _p not in sys.path and os.path.isdir(_p):
        sys.path.append(_p)

try:
    import antenv.axon_hooks  # noqa: F401
except ImportError:
    _hookmod = types.ModuleType("antenv.axon_hooks")
    _hookmod._hook = None
    _hookmod.set_axon_ntff_profile_hook = lambda h: setattr(_hookmod, "_hook", h)
    _hookmod.get_axon_ntff_profile_hook = lambda: _hookmod._hook
    sys.modules["antenv.axon_hooks"] = _hookmod

B, C, H, W = 4, 64, 96, 96
K = 9
Co = 64
HW = H * W                  # 9216
N_CORES = 8
HHALF = 48
NPIX = HHALF * W            # 4608 output pixels per core
NT = NPIX // 128            # 36 tiles
G0 = 4                      # tiles in the fast-path first group
NCH = K * 2                 # 18 contraction chunks of 128 = (k, l) x (v, c)

_CACHE = {}


def _build_module():
    from contextlib import ExitStack

    import concourse.bass as bass
    import concourse.tile as tile
    from concourse import bacc, mybir
    from concourse.bass_interp import get_hw_module
    from concourse.masks import make_identity

    f32 = mybir.dt.float32
    bf16 = mybir.dt.bfloat16
    i32 = mybir.dt.int32
    Alu = mybir.AluOpType
    Act = mybir.ActivationFunctionType

    nc = bacc.Bacc("TRN2", target_bir_lowering=False, debug=False,
                   enable_asserts=False, num_devices=N_CORES)

    # imgPad[r, 0:64] = pixel(r-97), imgPad[r, 64:128] = pixel(r-1), bf16.
    img_pad_ap = nc.dram_tensor("imgpad", [HW + 98, 2 * C], bf16,
                                kind="ExternalInput").ap()
    offmask_ap = nc.dram_tensor("offmask", [27, NPIX], f32, kind="ExternalInput").ap()
    byx_ap = nc.dram_tensor("byx", [128, NT * K * 2], f32, kind="ExternalInput").ap()
    w4_ap = nc.dram_tensor("w4", [(NCH + 1) * 128, Co], bf16, kind="ExternalInput").ap()
    bias_ap = nc.dram_tensor("biasv", [Co, 1], f32, kind="ExternalInput").ap()
    out_ap = nc.dram_tensor("out", [Co, NPIX], f32, kind="ExternalOutput").ap()

    with tile.TileContext(nc) as tc:
        with ExitStack() as ctx:
            cpool = ctx.enter_context(tc.tile_pool(name="consts", bufs=1))
            prep = ctx.enter_context(tc.tile_pool(name="prep", bufs=1))
            tp_ps = ctx.enter_context(tc.tile_pool(name="tr_ps", bufs=2, space="PSUM"))
            tb_ps = ctx.enter_context(tc.tile_pool(name="trB_ps", bufs=4, space="PSUM"))
            opsum = ctx.enter_context(tc.tile_pool(name="opsum", bufs=2, space="PSUM"))
            gpool = ctx.enter_context(tc.tile_pool(name="gather", bufs=8))
            wgpool = ctx.enter_context(tc.tile_pool(name="wg", bufs=4))
            stpool = ctx.enter_context(tc.tile_pool(name="sampT", bufs=3))
            obpool = ctx.enter_context(tc.tile_pool(name="ob", bufs=3))

            # ---- constants / inputs ----
            om = prep.tile([27, NPIX], f32)
            # first group's offset/mask chunk first: it gates the first gather
            nc.sync.dma_start(out=om[:, 0:G0 * 128],
                              in_=offmask_ap[:, 0:G0 * 128])
            ident = cpool.tile([128, 128], f32)
            make_identity(nc, ident[:])
            ident_bf = cpool.tile([128, 128], bf16)
            make_identity(nc, ident_bf[:])
            byx_sb = cpool.tile([128, NT * K * 2], f32)
            nc.sync.dma_start(out=byx_sb[:], in_=byx_ap)
            nc.sync.dma_start(out=om[:, G0 * 128:NPIX],
                              in_=offmask_ap[:, G0 * 128:NPIX])
            w4_sb = cpool.tile([128, (NCH + 1) * Co], bf16)
            nc.sync.dma_start(
                out=w4_sb[:].rearrange("p (f c) -> p f c", f=NCH + 1),
                in_=w4_ap.rearrange("(f p) c -> p f c", p=128),
            )
            ones_row = cpool.tile([128, 128], bf16)
            nc.vector.memset(ones_row[:], 0.0)
            nc.vector.memset(ones_row[0:1, :], 1.0)

            # DVE ops that can enter 2-port SBUF perf mode (tensor_scalar,
            # tensor_copy/cast) contend with GpSimd SWDGE descriptor writes;
            # steady-state vector work uses tensor_tensor against free-dim
            # broadcast constants instead.
            cst = cpool.tile([128, 8], f32)
            CF0, CF95, CFN1, CF94, CF97, CF9312 = range(6)
            for j, val in ((CF0, 0.0), (CF95, 95.0), (CFN1, -1.0),
                           (CF94, 94.0), (CF97, 97.0), (CF9312, 9312.0)):
                nc.vector.memset(cst[:, j:j + 1], val)
            cst_i = cpool.tile([128, 1], i32)
            nc.vector.memset(cst_i[:], 0)

            def cbc(j, shape):
                t = cst[:, j:j + 1]
                while len(t.shape) < len(shape):
                    t = t.unsqueeze(len(t.shape))
                return t.to_broadcast(list(shape))

            def cbci(shape):
                t = cst_i[:, 0:1]
                while len(t.shape) < len(shape):
                    t = t.unsqueeze(len(t.shape))
                return t.to_broadcast(list(shape))

            # ---- phase A2: offset/mask tile transposes (pixel-major) ----
            omT = prep.tile([128, NT * 27], f32)

            def a2(t0, t1):
                for t in range(t0, t1):
                    pt = tp_ps.tile([128, 27], f32, tag="tr")
                    nc.tensor.transpose(
                        out=pt[:], in_=om[:, t * 128:(t + 1) * 128],
                        identity=ident[:27, :27])
                    nc.scalar.activation(
                        out=omT[:, t * 27:(t + 1) * 27], in_=pt[:], func=Act.Copy)

            # ---- phase A3: index & weight math over tiles [t0, t1) ----
            omT3 = omT[:].rearrange("p (t c) -> p t c", t=NT)
            byx4 = byx_sb[:].rearrange("p (t k s) -> p t k s", t=NT, k=K)

            def t3(name):
                t = prep.tile([128, NT * K], f32, tag=name)
                return t, t[:].rearrange("p (t k) -> p t k", t=NT)

            py, pyv = t3("py")
            px, pxv = t3("px")
            fy, fyv = t3("fy")
            fx, fxv = t3("fx")
            wy, wyv = t3("wy")
            wx, wxv = t3("wx")
            ta, tav = t3("ta")
            tb, tbv = t3("tb")
            ti = prep.tile([128, NT * K], i32, tag="ti")
            tiv = ti[:].rearrange("p (t k) -> p t k", t=NT)
            vm0, vm0v = t3("vm0")
            vm1, vm1v = t3("vm1")
            vc0, vc0v = t3("vc0")
            vc1, vc1v = t3("vc1")
            cA, cAv = t3("cA")
            cB, cBv = t3("cB")
            wq = prep.tile([128, NT * K * 4], f32)
            wq5 = wq[:].rearrange("p (t k l v) -> p t k l v", t=NT, k=K, l=2)
            idxf = prep.tile([128, NT * K], f32)
            idxi = prep.tile([128, NT * K], i32)
            idxi3 = idxi[:].rearrange("p (t k) -> p t k", t=NT)

            V = nc.vector

            def a3(t0, t1):
                s = slice(t0, t1)
                f = slice(t0 * K, t1 * K)
                dyv = omT3[:, s, 0:18:2]      # [128, nt, 9]
                dxv = omT3[:, s, 1:18:2]
                mv = omT3[:, s, 18:27]
                hov = byx4[:, s, :, 0]        # ho - 1 + ky
                wov = byx4[:, s, :, 1]        # wo - 1 + kx
                _py, _px = pyv[:, s], pxv[:, s]
                _fy, _fx = fyv[:, s], fxv[:, s]
                _wy, _wx = wyv[:, s], wxv[:, s]
                _ta, _tb, _ti = tav[:, s], tbv[:, s], tiv[:, s]
                _vm0, _vm1 = vm0v[:, s], vm1v[:, s]
                _vc0, _vc1 = vc0v[:, s], vc1v[:, s]
                _cA, _cB = cAv[:, s], cBv[:, s]
                shp = list(_py.shape)
                zf = cbc(CF0, shp)

                # py = dy + (ho - 1 + ky); floor & frac (cast-roundtrip floor,
                # robust to any int rounding mode; casts via tensor_tensor
                # add-zero keep DVE in 1-port mode)
                V.tensor_tensor(out=_py, in0=dyv, in1=hov, op=Alu.add)
                V.tensor_tensor(out=_ti, in0=_py, in1=zf, op=Alu.add)
                V.tensor_tensor(out=_ta, in0=_ti, in1=cbci(shp), op=Alu.add)
                V.tensor_tensor(out=_tb, in0=_ta, in1=_py, op=Alu.is_gt)
                V.tensor_tensor(out=_fy, in0=_ta, in1=_tb, op=Alu.subtract)
                V.tensor_tensor(out=_wy, in0=_py, in1=_fy, op=Alu.subtract)
                # px = dx + (wo - 1 + kx)
                V.tensor_tensor(out=_px, in0=dxv, in1=wov, op=Alu.add)
                V.tensor_tensor(out=_ti, in0=_px, in1=zf, op=Alu.add)
                V.tensor_tensor(out=_ta, in0=_ti, in1=cbci(shp), op=Alu.add)
                V.tensor_tensor(out=_tb, in0=_ta, in1=_px, op=Alu.is_gt)
                V.tensor_tensor(out=_fx, in0=_ta, in1=_tb, op=Alu.subtract)
                V.tensor_tensor(out=_wx, in0=_px, in1=_fx, op=Alu.subtract)

                # row validity (* mask) and column validity
                V.tensor_tensor(out=_ta, in0=_fy, in1=zf, op=Alu.is_ge)
                V.tensor_tensor(out=_tb, in0=_fy, in1=cbc(CF95, shp), op=Alu.is_le)
                V.tensor_tensor(out=_vm0, in0=_ta, in1=_tb, op=Alu.mult)
                V.tensor_tensor(out=_vm0, in0=_vm0, in1=mv, op=Alu.mult)
                V.tensor_tensor(out=_ta, in0=_fy, in1=cbc(CFN1, shp), op=Alu.is_ge)
                V.tensor_tensor(out=_tb, in0=_fy, in1=cbc(CF94, shp), op=Alu.is_le)
                V.tensor_tensor(out=_vm1, in0=_ta, in1=_tb, op=Alu.mult)
                V.tensor_tensor(out=_vm1, in0=_vm1, in1=mv, op=Alu.mult)
                V.tensor_tensor(out=_ta, in0=_fx, in1=zf, op=Alu.is_ge)
                V.tensor_tensor(out=_tb, in0=_fx, in1=cbc(CF95, shp), op=Alu.is_le)
                V.tensor_tensor(out=_vc0, in0=_ta, in1=_tb, op=Alu.mult)
                V.tensor_tensor(out=_ta, in0=_fx, in1=cbc(CFN1, shp), op=Alu.is_ge)
                V.tensor_tensor(out=_tb, in0=_fx, in1=cbc(CF94, shp), op=Alu.is_le)
                V.tensor_tensor(out=_vc1, in0=_ta, in1=_tb, op=Alu.mult)

                # bilinear coefficients: cy0/cy1 (carry mask), cx0/cx1
                nc.scalar.activation(out=_ta, in_=_wy, func=Act.Copy, bias=1.0, scale=-1.0)
                V.tensor_tensor(out=_cA, in0=_ta, in1=_vm0, op=Alu.mult)   # cy0
                V.tensor_tensor(out=_cB, in0=_wy, in1=_vm1, op=Alu.mult)   # cy1
                nc.scalar.activation(out=_ta, in_=_wx, func=Act.Copy, bias=1.0, scale=-1.0)
                V.tensor_tensor(out=_vc0, in0=_ta, in1=_vc0, op=Alu.mult)  # cx0
                V.tensor_tensor(out=_vc1, in0=_wx, in1=_vc1, op=Alu.mult)  # cx1

                V.tensor_tensor(out=wq5[:, s, :, 0, 0], in0=_cA, in1=_vc0, op=Alu.mult)
                V.tensor_tensor(out=wq5[:, s, :, 0, 1], in0=_cB, in1=_vc0, op=Alu.mult)
                V.tensor_tensor(out=wq5[:, s, :, 1, 0], in0=_cA, in1=_vc1, op=Alu.mult)
                V.tensor_tensor(out=wq5[:, s, :, 1, 1], in0=_cB, in1=_vc1, op=Alu.mult)

                # quad-gather indices: clamp(96*fy + fx + 97, 0, 9312)
                idxfv = idxf[:].rearrange("p (t k) -> p t k", t=NT)[:, s]
                V.scalar_tensor_tensor(out=idxfv, in0=_fy, scalar=96.0, in1=_fx,
                                       op0=Alu.mult, op1=Alu.add)
                fl = [128, (t1 - t0) * K]
                V.tensor_tensor(out=idxf[:, f], in0=idxf[:, f],
                                in1=cbc(CF97, fl), op=Alu.add)
                V.tensor_tensor(out=idxf[:, f], in0=idxf[:, f],
                                in1=cbc(CF0, fl), op=Alu.max)
                V.tensor_tensor(out=idxf[:, f], in0=idxf[:, f],
                                in1=cbc(CF9312, fl), op=Alu.min)
                V.tensor_tensor(out=idxi[:, f], in0=idxf[:, f],
                                in1=cbc(CF0, fl), op=Alu.add)

            a2(0, G0)
            a3(0, G0)
            a2(G0, 12)
            a3(G0, 12)
            a2(12, NT)
            a3(12, NT)

            wqv_all = wq[:].rearrange("p (t r) -> p t r", t=NT)
            w4v = w4_sb[:].rearrange("p (f c) -> p f c", f=NCH + 1)

            # ---- phase B ----
            for t in range(NT):
                g = gpool.tile([128, K * 4 * C], bf16)
                for k in range(K):
                    nc.gpsimd.indirect_dma_start(
                        out=g[:, k * 4 * C:(k + 1) * 4 * C],
                        out_offset=None,
                        in_=img_pad_ap,
                        in_offset=bass.IndirectOffsetOnAxis(
                            ap=idxi3[:, t, k:k + 1], axis=0),
                    )
                wg = wgpool.tile([128, K * 4 * C], bf16)
                # weighting on the scalar engine (per-partition scale AP), one
                # [128, 64] scale-copy per (tap, l, v): DVE stays off the
                # shared SBUF port so Q7 SWDGE generation never blocks on it
                for q in range(K * 4):
                    nc.scalar.activation(
                        out=wg[:, q * C:(q + 1) * C],
                        in_=g[:, q * C:(q + 1) * C], func=Act.Copy,
                        scale=wqv_all[:, t, q:q + 1])

                wgT = stpool.tile([128, NCH * 128], bf16)
                for ci in range(NCH):
                    pt = tb_ps.tile([128, 128], bf16, tag="trB")
                    nc.tensor.transpose(out=pt[:], in_=wg[:, ci * 128:(ci + 1) * 128],
                                        identity=ident_bf[:])
                    V.tensor_copy(
                        out=wgT[:, ci * 128:(ci + 1) * 128], in_=pt[:])

                po = opsum.tile([Co, 128], f32)
                for ci in range(NCH):
                    nc.tensor.matmul(
                        out=po[:], lhsT=w4v[:, ci, :],
                        rhs=wgT[:, ci * 128:(ci + 1) * 128],
                        start=(ci == 0), stop=False)
                nc.tensor.matmul(
                    out=po[:], lhsT=w4v[:, NCH, :], rhs=ones_row[:],
                    start=False, stop=True)

                ob = obpool.tile([Co, 128], f32)
                V.tensor_copy(out=ob[:], in_=po[:])
                nc.sync.dma_start(out=out_ap[:, t * 128:(t + 1) * 128], in_=ob[:])

    nc.compile()
    nc.m = get_hw_module(nc.m)
    return nc


def _host_prep(input, offset, mask, weight, bias):
    f32 = np.float32
    bf16 = ml_dtypes.bfloat16
    input = np.ascontiguousarray(input, dtype=f32)
    offset = np.ascontiguousarray(offset, dtype=f32)
    mask = np.ascontiguousarray(mask, dtype=f32)
    weight = np.ascontiguousarray(weight, dtype=f32)
    bias = np.ascontiguousarray(bias, dtype=f32)

    # weight [Co, C, 3, 3] -> W4[(k, l, v, c), co] bf16: conv weight
    # replicated over the 4 bilinear quad slots (l = x-side, v = y-side),
    # matching the gathered quad layout [v00,v10 | v01,v11] per tap.
    wr = weight.reshape(Co, C, K)                     # [co, c, k]
    wkc = np.transpose(wr, (2, 1, 0))                 # [k, c, co]
    w4 = np.broadcast_to(wkc[:, None, None, :, :], (K, 2, 2, C, Co))
    w4 = w4.reshape(NCH * 128, Co)
    # chunk NCH: bias as an outer product against a ones-row rhs
    w4b = np.zeros((128, Co), dtype=np.float32)
    w4b[0, :] = bias
    w4 = np.ascontiguousarray(np.concatenate([w4, w4b], axis=0), dtype=bf16)

    biasv = bias.reshape(Co, 1)
    kyv = (np.arange(K, dtype=f32) // 3)
    kxv = (np.arange(K, dtype=f32) % 3)

    pix = np.arange(NPIX).reshape(NT, 128)
    in_maps = []
    imgpads = {}
    for core in range(N_CORES):
        b, h = core // 2, core % 2
        ho0 = h * HHALF
        ho = ho0 + pix // W
        wo = pix % W
        base_y = (ho - 1)[:, :, None] + kyv[None, None, :]   # [NT, 128, K]
        base_x = (wo - 1)[:, :, None] + kxv[None, None, :]
        byx = np.stack([base_y, base_x], axis=-1)            # [NT, 128, K, 2]
        byx = np.ascontiguousarray(
            byx.transpose(1, 0, 2, 3).reshape(128, NT * K * 2), dtype=f32)
        offmask = np.concatenate(
            [offset[b, :, ho0:ho0 + HHALF, :].reshape(18, NPIX),
             mask[b, :, ho0:ho0 + HHALF, :].reshape(K, NPIX)], axis=0)
        # quad-packed padded image: imgPad[r] = [pixel(r-97) | pixel(r-1)];
        # shared between the two cores of a batch.
        if b not in imgpads:
            imgT = input[b].reshape(C, HW).T.astype(bf16)    # [HW, C]
            ip = np.zeros((HW + 98, 2 * C), dtype=bf16)
            ip[97:97 + HW, 0:C] = imgT
            ip[1:1 + HW, C:2 * C] = imgT
            imgpads[b] = ip
        in_maps.append({
            "imgpad": imgpads[b],
            "offmask": np.ascontiguousarray(offmask),
            "byx": byx,
            "w4": w4,
            "biasv": biasv,
        })
    return in_maps


def kernel(input, offset, mask, weight, bias):
    from concourse.bass_utils import run_bass_kernel_spmd

    if "nc" not in _CACHE:
        _CACHE["nc"] = _build_module()
    nc = _CACHE["nc"]

    in_maps = _host_prep(input, offset, mask, weight, bias)
    res = run_bass_kernel_spmd(nc, in_maps, core_ids=list(range(N_CORES)))

    out = np.empty((B, Co, H, W), dtype=np.float32)
    for core in range(N_CORES):
        b, h = core // 2, core % 2
        ho0 = h * HHALF
        out[b, :, ho0:ho0 + HHALF, :] = \
            res.results[core]["out"].reshape(Co, HHALF, W)
    return out
